# revision 20
# baseline (speedup 1.0000x reference)
"""TRN2 kernel for nn_End2EndRoialign (nms_detection).

Pipeline: decode scores on 8 NeuronCores (data-parallel over the 102000
anchors, the bandwidth-heavy stage: 47.7 MB input read), then the tiny
discrete stages (top-1000, class-aware greedy NMS on 1000 candidates,
top-100) as an exact float32 replica on host, then ROIAlign + mask
linear-combination (matmul-heavy stage) -- device when ROI kernel enabled,
host BLAS fallback otherwise.

Self-contained: only imports numpy + the concourse (bass) runtime that ships
with the environment. All shapes hardcoded from the problem spec:
  x     [1, 102000, 117] f32
  proto [1, 32, 320, 320] f32
Returns (dets [100,6] f32, masks [100,56,56] f32) like the reference.
"""

import os
import numpy as np

# ---- problem constants (hardcoded) ----
N_ANCH = 102000
NC = 80
NM = 32
TOPK = 1000
MAX_OBJ = 100
IOU_THRES = np.float32(0.45)
SCORE_THRES = np.float32(0.25)
MAX_WH = np.float32(1280.0)
MASK_RES = 56
SR = 2
PROTO_HW = 320

N_CORES = 8
PER_CORE = N_ANCH // N_CORES        # 12750
SLOTS = 12800                       # 128 partitions x 100 anchors
TPP = SLOTS // 128                  # 100 anchors per partition
D = 117

_f32 = np.float32

# wall time spent inside device launches on the last kernel() call (ns);
# upper bound on HW exec time (includes host<->device transfer + dispatch)
LAST_DEVICE_WALL_NS = 0

# ---------------------------------------------------------------------------
# Device kernel A: per-anchor score = obj * max_c cls  (exact: fp32 rounding
# of obj*c is monotone in c, so max_c round(obj*c) == round(obj * max_c c)).
# ---------------------------------------------------------------------------
_DECODE = None


def _build_decode():
    import concourse.bass as bass
    import concourse.mybir as mybir
    from contextlib import ExitStack

    f32 = mybir.dt.float32
    nc = bass.Bass()
    x_in = nc.dram_tensor("xs", [SLOTS, D], f32, kind="ExternalInput")
    s_out = nc.dram_tensor("scores", [128, TPP], f32, kind="ExternalOutput")

    NCHUNK = 4
    T = TPP // NCHUNK  # 25 anchors/partition per chunk
    src = x_in[:].rearrange("(p t) d -> p (t d)", p=128)  # [128, 11700]

    with ExitStack() as ctx:
        xts = [ctx.enter_context(nc.sbuf_tensor(f"xt{i}", [128, T * D], f32))
               for i in range(2)]
        mx = ctx.enter_context(nc.sbuf_tensor("mx", [128, T], f32))
        st = ctx.enter_context(nc.sbuf_tensor("st", [128, TPP], f32))
        dma_sem = ctx.enter_context(nc.semaphore())
        v_sem = ctx.enter_context(nc.semaphore())
        r_sem = ctx.enter_context(nc.semaphore())
        block = ctx.enter_context(nc.Block())

        @block.sync
        def _(sync):
            for k in range(NCHUNK):
                if k >= 2:
                    # buffer k%2 free once compute of chunk k-2 finished
                    sync.wait_ge(v_sem, k - 1)
                sync.dma_start(
                    xts[k % 2][:], src[:, k * T * D:(k + 1) * T * D]
                ).then_inc(dma_sem, 16)
            sync.wait_ge(v_sem, NCHUNK)
            sync.dma_start(s_out[:], st[:]).then_inc(dma_sem, 16)

        @block.vector
        def _(vector):
            for k in range(NCHUNK):
                vector.wait_ge(dma_sem, 16 * (k + 1))
                xt = xts[k % 2]
                x3 = xt[:].rearrange("p (t d) -> p t d", d=D)
                vector.tensor_reduce(
                    mx[:], x3[:, :, 5:5 + NC],
                    axis=mybir.AxisListType.X, op=mybir.AluOpType.max,
                ).then_inc(r_sem, 1)
                # same-engine RAW: reduce tail must land before mul reads mx
                vector.wait_ge(r_sem, k + 1)
                obj = xt[:, 4:T * D:D]
                vector.tensor_mul(
                    st[:, k * T:(k + 1) * T], mx[:], obj
                ).then_inc(v_sem, 1)
    return nc


def _device_scores(x0):
    """x0: [102000,117] f32 -> scores [102000] f32 computed on 8 cores."""
    global _DECODE
    from concourse.bass_utils import run_bass_kernel_spmd

    if _DECODE is None:
        _DECODE = _build_decode()
    in_maps = []
    for c in range(N_CORES):
        a = c * PER_CORE
        if a + SLOTS <= N_ANCH:
            xs = x0[a:a + SLOTS]
        else:
            xs = np.concatenate(
                [x0[a:], np.zeros((a + SLOTS - N_ANCH, D), np.float32)], 0
            )
        in_maps.append({"xs": np.ascontiguousarray(xs)})
    import time
    global LAST_DEVICE_WALL_NS
    t0 = time.perf_counter()
    res = run_bass_kernel_spmd(_DECODE, in_maps, list(range(N_CORES)))
    LAST_DEVICE_WALL_NS += int((time.perf_counter() - t0) * 1e9)
    outs = res.results
    return np.concatenate(
        [np.asarray(outs[c]["scores"]).reshape(-1)[:PER_CORE] for c in range(N_CORES)]
    )


def _host_scores(x0):
    return x0[:, 4] * np.max(x0[:, 5:5 + NC], axis=1)


# ---------------------------------------------------------------------------
# Host: exact fp32 replicas of the discrete stages
# ---------------------------------------------------------------------------

def _box_iou_np(a):
    area = (a[:, 2] - a[:, 0]) * (a[:, 3] - a[:, 1])
    lt = np.maximum(a[:, None, :2], a[None, :, :2])
    rb = np.minimum(a[:, None, 2:], a[None, :, 2:])
    wh = np.clip(rb - lt, _f32(0.0), None)
    inter = wh[..., 0] * wh[..., 1]
    return inter / (area[:, None] + area[None, :] - inter + _f32(1e-9))


def _greedy_nms(iou, keep0):
    keep = keep0.copy()
    rng = np.arange(TOPK)
    for i in range(TOPK):
        if keep[i]:
            sup = (iou[i] > IOU_THRES) & (rng > i)
            keep &= ~sup
            keep[i] = True
    return keep


def _interp_matrix(b0, blen):
    """[56,320] pooled bilinear-interp matrix for one ROI axis (exact fp32)."""
    P = MASK_RES * SR
    t = (np.arange(P, dtype=np.float32) + _f32(0.5)) / _f32(P)
    xs = b0 + t * blen
    v = ((xs > _f32(-1.0)) & (xs < _f32(PROTO_HW))).astype(np.float32)
    xc = np.clip(xs, _f32(0.0), _f32(PROTO_HW - 1))
    x0 = np.floor(xc).astype(np.int32)
    x1 = np.minimum(x0 + 1, PROTO_HW - 1)
    lx = xc - x0.astype(np.float32)
    S = np.zeros((P, PROTO_HW), np.float32)
    rows = np.arange(P)
    np.add.at(S, (rows, x0), (_f32(1.0) - lx) * v)
    np.add.at(S, (rows, x1), lx * v)
    return _f32(0.5) * (S[0::2] + S[1::2])


def _build_interp_mats(boxes_f):
    """boxes_f [100,4] -> Ay, Ax each [100,56,320] f32."""
    Ay = np.empty((MAX_OBJ, MASK_RES, PROTO_HW), np.float32)
    Ax = np.empty((MAX_OBJ, MASK_RES, PROTO_HW), np.float32)
    PS = _f32(0.25)
    for n in range(MAX_OBJ):
        b = boxes_f[n] * PS
        rw = np.maximum(b[2] - b[0], _f32(1.0))
        rh = np.maximum(b[3] - b[1], _f32(1.0))
        Ax[n] = _interp_matrix(b[0], rw)
        Ay[n] = _interp_matrix(b[1], rh)
    return Ay, Ax


# ---------------------------------------------------------------------------
# Device kernel B: per-ROI mask = sigmoid(Ay @ (coeff . proto) @ Ax^T) * vf
# ROI-sharded: 13 ROIs per core (100 padded to 104). Mixing contracts the 32
# proto channels on PE (proto-stationary, [32,128] weight tiles), interp is
# two small matmuls per ROI with a PE-transpose between them.
# ---------------------------------------------------------------------------
_ROIK = None
NROI = 13  # per core


def _build_roik():
    import concourse.bass as bass
    import concourse.mybir as mybir
    from contextlib import ExitStack

    f32 = mybir.dt.float32
    nc = bass.Bass()
    proto_in = nc.dram_tensor("proto", [32, 320 * 320], f32, kind="ExternalInput")
    coefft_in = nc.dram_tensor("coefft", [32, NROI], f32, kind="ExternalInput")
    axt_in = nc.dram_tensor("axt", [128, NROI * 3 * 56], f32, kind="ExternalInput")
    ayt_in = nc.dram_tensor("ayt", [128, NROI * 3 * 56], f32, kind="ExternalInput")
    vfb_in = nc.dram_tensor("vfb", [56, NROI], f32, kind="ExternalInput")
    ident_in = nc.dram_tensor("ident", [128, 128], f32, kind="ExternalInput")
    out_m = nc.dram_tensor("masksT", [NROI, 56, 56], f32, kind="ExternalOutput")

    NG = 10          # proto row-groups
    GH = 32          # h rows per group
    with ExitStack() as ctx:
        pt = [ctx.enter_context(nc.sbuf_tensor(f"pt{i}", [32, GH * 320], f32))
              for i in range(2)]
        mixedT = ctx.enter_context(
            nc.sbuf_tensor("mixedT", [128, NROI * 3 * 320], f32))
        axt_sb = ctx.enter_context(
            nc.sbuf_tensor("axt_sb", [128, NROI * 3 * 56], f32))
        ayt_sb = ctx.enter_context(
            nc.sbuf_tensor("ayt_sb", [128, NROI * 3 * 56], f32))
        coefft_sb = ctx.enter_context(nc.sbuf_tensor("coefft_sb", [32, NROI], f32))
        vfb_sb = ctx.enter_context(nc.sbuf_tensor("vfb_sb", [56, NROI], f32))
        ident_sb = ctx.enter_context(nc.sbuf_tensor("ident_sb", [128, 128], f32))
        t1x_sb = ctx.enter_context(nc.sbuf_tensor("t1x_sb", [56, 320], f32))
        t1xt_sb = ctx.enter_context(nc.sbuf_tensor("t1xt_sb", [128, 3 * 56], f32))
        mk_sb = ctx.enter_context(nc.sbuf_tensor("mk_sb", [56, NROI * 56], f32))

        pm = [ctx.enter_context(nc.psum_tensor(f"pm{i}", [128, GH * NROI], f32))
              for i in range(2)]
        pt1 = ctx.enter_context(nc.psum_tensor("ps_t1", [56, 320], f32))
        ptr = ctx.enter_context(nc.psum_tensor("ps_tr", [128, 56], f32))
        pt2 = ctx.enter_context(nc.psum_tensor("ps_t2", [56, 56], f32))

        ds = ctx.enter_context(nc.semaphore())        # dma completions
        ms_sem = ctx.enter_context(nc.semaphore())    # memsets done
        pe_mix = ctx.enter_context(nc.semaphore())    # PE mixing units
        dve_mix = ctx.enter_context(nc.semaphore())   # DVE mixing copies
        pe_t1 = ctx.enter_context(nc.semaphore())
        dve_t1 = ctx.enter_context(nc.semaphore())
        pe_tr = ctx.enter_context(nc.semaphore())
        dve_tr = ctx.enter_context(nc.semaphore())
        pe_t2 = ctx.enter_context(nc.semaphore())
        act_s = ctx.enter_context(nc.semaphore())
        dve_vf = ctx.enter_context(nc.semaphore())
        block = ctx.enter_context(nc.Block())

        mixedT4 = mixedT[:].rearrange("p (n c w) -> p n c w", n=NROI, c=3)
        axt4 = axt_sb[:].rearrange("p (n c f) -> p n c f", n=NROI, c=3)
        ayt4 = ayt_sb[:].rearrange("p (n c f) -> p n c f", n=NROI, c=3)

        @block.sync
        def _(sync):
            sync.dma_start(coefft_sb[:], coefft_in[:]).then_inc(ds, 16)
            sync.dma_start(axt_sb[:], axt_in[:]).then_inc(ds, 16)
            sync.dma_start(ayt_sb[:], ayt_in[:]).then_inc(ds, 16)
            sync.dma_start(vfb_sb[:], vfb_in[:]).then_inc(ds, 16)
            sync.dma_start(ident_sb[:], ident_in[:]).then_inc(ds, 16)
            for g in range(NG):
                if g >= 2:
                    sync.wait_ge(pe_mix, 3 * (g - 1))
                sync.dma_start(
                    pt[g % 2][:], proto_in[:, g * GH * 320:(g + 1) * GH * 320]
                ).then_inc(ds, 16)
            sync.wait_ge(dve_vf, NROI)
            out_r = out_m[:].rearrange("n p q -> p n q")
            mk_r = mk_sb[:].rearrange("p (n q) -> p n q", n=NROI)
            sync.dma_start(out_r, mk_r).then_inc(ds, 16)

        @block.tensor
        def _(tensor):
            # ---- mixing: mixedT[w_part, n, wb, h] = sum_c proto[c,h,w]*coeff[n,c]
            tensor.wait_ge(ds, 16)  # coefft
            u = 0
            for g in range(NG):
                tensor.wait_ge(ds, 16 * (6 + g))
                for wb in range(3):
                    wcols = 128 if wb < 2 else 64
                    if u >= 2:
                        tensor.wait_ge(dve_mix, u - 1)
                    for h in range(GH):
                        lhsT = pt[g % 2][:, h * 320 + wb * 128:
                                         h * 320 + wb * 128 + wcols]
                        mm = tensor.matmul(
                            pm[u % 2][:wcols, h * NROI:(h + 1) * NROI],
                            lhsT, coefft_sb[:], start=True, stop=True,
                        )
                    mm.then_inc(pe_mix, 1)
                    u += 1
            # ---- per-ROI interp
            tensor.wait_ge(dve_mix, 30)   # mixedT fully assembled
            tensor.wait_ge(ms_sem, 2)
            for n in range(NROI):
                if n > 0:
                    tensor.wait_ge(dve_t1, n)   # pt1 free
                for wb in range(3):
                    mmi = tensor.matmul(
                        pt1[:], axt4[:, n, wb, :], mixedT4[:, n, wb, :],
                        start=(wb == 0), stop=(wb == 2),
                    )
                mmi.then_inc(pe_t1, 1)
                tensor.wait_ge(dve_t1, n + 1)   # t1x_sb ready
                for hb in range(3):
                    rows = 128 if hb < 2 else 64
                    if n > 0 or hb > 0:
                        tensor.wait_ge(dve_tr, n * 3 + hb)  # ptr free
                    tensor.transpose(
                        ptr[:rows, :], t1x_sb[:, hb * 128:hb * 128 + rows],
                        ident_sb[:56, :56],
                    ).then_inc(pe_tr, 1)
                tensor.wait_ge(dve_tr, n * 3 + 3)  # t1xt_sb ready
                if n > 0:
                    tensor.wait_ge(act_s, n)       # pt2 free
                for hc in range(3):
                    mmj = tensor.matmul(
                        pt2[:], t1xt_sb[:, hc * 56:(hc + 1) * 56],
                        ayt4[:, n, hc, :],
                        start=(hc == 0), stop=(hc == 2),
                    )
                mmj.then_inc(pe_t2, 1)

        @block.vector
        def _(vector):
            vector.memset(mixedT[:], 0.0).then_inc(ms_sem, 1)
            vector.memset(t1xt_sb[:], 0.0).then_inc(ms_sem, 1)
            vector.wait_ge(ms_sem, 2)
            u = 0
            for g in range(NG):
                for wb in range(3):
                    wcols = 128 if wb < 2 else 64
                    vector.wait_ge(pe_mix, u + 1)
                    src = pm[u % 2][:wcols, :].rearrange(
                        "p (h n) -> p n h", n=NROI)
                    dst = mixedT4[:wcols, :, wb, g * GH:(g + 1) * GH]
                    vector.tensor_copy(dst, src).then_inc(dve_mix, 1)
                    u += 1
            for n in range(NROI):
                vector.wait_ge(pe_t1, n + 1)
                vector.tensor_copy(t1x_sb[:], pt1[:]).then_inc(dve_t1, 1)
                for hb in range(3):
                    rows = 128 if hb < 2 else 64
                    vector.wait_ge(pe_tr, n * 3 + hb + 1)
                    vector.tensor_copy(
                        t1xt_sb[:rows, hb * 56:(hb + 1) * 56], ptr[:rows, :]
                    ).then_inc(dve_tr, 1)
                # vf multiply after sigmoid
                vector.wait_ge(act_s, n + 1)
                vector.tensor_scalar_mul(
                    mk_sb[:, n * 56:(n + 1) * 56],
                    mk_sb[:, n * 56:(n + 1) * 56],
                    vfb_sb[:, n:n + 1],
                ).then_inc(dve_vf, 1)

        @block.scalar
        def _(scalar):
            for n in range(NROI):
                scalar.wait_ge(pe_t2, n + 1)
                scalar.activation(
                    mk_sb[:, n * 56:(n + 1) * 56], pt2[:],
                    mybir.ActivationFunctionType.Sigmoid,
                ).then_inc(act_s, 1)
    return nc


def _host_masks(proto0, coeff_f, vf, Ay, Ax):
    mixed = coeff_f.astype(np.float32) @ proto0.reshape(NM, -1)  # [100, 102400]
    masks = np.empty((MAX_OBJ, MASK_RES, MASK_RES), np.float32)
    for n in range(MAX_OBJ):
        m = mixed[n].reshape(PROTO_HW, PROTO_HW)
        logits = Ay[n] @ m @ Ax[n].T
        masks[n] = vf[n] / (_f32(1.0) + np.exp(-logits))
    return masks


def _pack_interp(A):
    """A [104,56,320] -> per-core [8][128, 13*3*56] transposed/padded blocks."""
    Ap = np.zeros((8 * NROI, MASK_RES, 384), np.float32)
    Ap[:A.shape[0], :, :PROTO_HW] = A
    At = Ap.transpose(0, 2, 1).reshape(8 * NROI, 3, 128, MASK_RES)
    out = []
    for c in range(8):
        blk = At[c * NROI:(c + 1) * NROI].transpose(2, 0, 1, 3)  # [128,13,3,56]
        out.append(np.ascontiguousarray(blk).reshape(128, NROI * 3 * 56))
    return out


def _device_masks(proto0, coeff_f, vf, Ay, Ax):
    global _ROIK
    from concourse.bass_utils import run_bass_kernel_spmd

    if _ROIK is None:
        _ROIK = _build_roik()
    NPAD = 8 * NROI
    cf = np.zeros((NPAD, NM), np.float32)
    cf[:MAX_OBJ] = coeff_f
    vfp = np.zeros(NPAD, np.float32)
    vfp[:MAX_OBJ] = vf
    Ayp = np.zeros((NPAD, MASK_RES, PROTO_HW), np.float32)
    Ayp[:MAX_OBJ] = Ay
    Axp = np.zeros((NPAD, MASK_RES, PROTO_HW), np.float32)
    Axp[:MAX_OBJ] = Ax
    ay_l = _pack_interp(Ayp)
    ax_l = _pack_interp(Axp)
    proto_flat = np.ascontiguousarray(proto0.reshape(NM, -1))
    ident = np.eye(128, dtype=np.float32)
    in_maps = []
    for c in range(8):
        sl = slice(c * NROI, (c + 1) * NROI)
        in_maps.append({
            "proto": proto_flat,
            "coefft": np.ascontiguousarray(cf[sl].T),
            "axt": ax_l[c],
            "ayt": ay_l[c],
            "vfb": np.ascontiguousarray(
                np.repeat(vfp[sl][None, :], MASK_RES, 0)),
            "ident": ident,
        })
    import time
    global LAST_DEVICE_WALL_NS
    t0 = time.perf_counter()
    res = run_bass_kernel_spmd(_ROIK, in_maps, list(range(8)))
    LAST_DEVICE_WALL_NS += int((time.perf_counter() - t0) * 1e9)
    masks = np.empty((MAX_OBJ, MASK_RES, MASK_RES), np.float32)
    for c in range(8):
        mt = np.asarray(res.results[c]["masksT"])  # [13,56,56] = logitsT
        for n in range(NROI):
            g = c * NROI + n
            if g < MAX_OBJ:
                masks[g] = mt[n].T
    return masks


# ---------------------------------------------------------------------------
# entry point
# ---------------------------------------------------------------------------

def kernel(x, proto):
    global LAST_DEVICE_WALL_NS
    LAST_DEVICE_WALL_NS = 0
    x0 = np.asarray(x[0], dtype=np.float32)          # [102000, 117]
    proto0 = np.asarray(proto[0], dtype=np.float32)  # [32, 320, 320]

    if os.environ.get("KERNEL_HOST_ONLY"):
        s = _host_scores(x0)
    else:
        try:
            s = _device_scores(x0)
        except Exception:
            s = _host_scores(x0)

    # top-1000 (desc, ties -> lower index, matching jax.lax.top_k)
    order = np.argsort(-s, kind="stable")[:TOPK].astype(np.int64)
    sc_k = s[order]
    pr = x0[order]
    cxcy, wh = pr[:, :2], pr[:, 2:4]
    boxes_k = np.concatenate(
        [cxcy - wh / _f32(2.0), cxcy + wh / _f32(2.0)], 1)
    sa = pr[:, 4:5] * pr[:, 5:5 + NC]
    cls_k = np.argmax(sa, axis=1).astype(np.int64)
    coeff_k = pr[:, 5 + NC:5 + NC + NM]

    off = boxes_k + (cls_k.astype(np.float32) * MAX_WH)[:, None]
    iou = _box_iou_np(off)
    keep0 = sc_k > SCORE_THRES
    keep = _greedy_nms(iou, keep0)

    sel = np.where(keep, sc_k, _f32(-1.0))
    fi = np.argsort(-sel, kind="stable")[:MAX_OBJ]
    valid = sel[fi] > SCORE_THRES
    vf = valid.astype(np.float32)
    boxes_f = boxes_k[fi] * vf[:, None]
    scores_f = sc_k[fi] * vf
    cls_f = cls_k[fi].astype(np.float32) * vf
    coeff_f = coeff_k[fi]

    Ay, Ax = _build_interp_mats(boxes_f)
    if os.environ.get("KERNEL_HOST_ONLY") or os.environ.get("KERNEL_HOST_MASKS"):
        masks = _host_masks(proto0, coeff_f, vf, Ay, Ax)
    else:
        try:
            masks = _device_masks(proto0, coeff_f, vf, Ay, Ax)
        except Exception:
            masks = _host_masks(proto0, coeff_f, vf, Ay, Ax)

    dets = np.concatenate(
        [boxes_f, scores_f[:, None], cls_f[:, None]], 1).astype(np.float32)
    return dets, masks


# revision 25
# speedup vs baseline: 1.4072x; 1.4072x over previous
"""TRN2 kernel for nn_End2EndRoialign (nms_detection).

Pipeline: decode scores on 8 NeuronCores (data-parallel over the 102000
anchors, the bandwidth-heavy stage: 47.7 MB input read), then the tiny
discrete stages (top-1000, class-aware greedy NMS on 1000 candidates,
top-100) as an exact float32 replica on host, then ROIAlign + mask
linear-combination (matmul-heavy stage) -- device when ROI kernel enabled,
host BLAS fallback otherwise.

Self-contained: only imports numpy + the concourse (bass) runtime that ships
with the environment. All shapes hardcoded from the problem spec:
  x     [1, 102000, 117] f32
  proto [1, 32, 320, 320] f32
Returns (dets [100,6] f32, masks [100,56,56] f32) like the reference.
"""

import os
import numpy as np

# ---- problem constants (hardcoded) ----
N_ANCH = 102000
NC = 80
NM = 32
TOPK = 1000
MAX_OBJ = 100
IOU_THRES = np.float32(0.45)
SCORE_THRES = np.float32(0.25)
MAX_WH = np.float32(1280.0)
MASK_RES = 56
SR = 2
PROTO_HW = 320

N_CORES = 8
PER_CORE = N_ANCH // N_CORES        # 12750
SLOTS = 12800                       # 128 partitions x 100 anchors
TPP = SLOTS // 128                  # 100 anchors per partition
D = 117

_f32 = np.float32

# wall time spent inside device launches on the last kernel() call (ns);
# upper bound on HW exec time (includes host<->device transfer + dispatch)
LAST_DEVICE_WALL_NS = 0

# ---------------------------------------------------------------------------
# Device kernel A: per-anchor score = obj * max_c cls  (exact: fp32 rounding
# of obj*c is monotone in c, so max_c round(obj*c) == round(obj * max_c c)).
# ---------------------------------------------------------------------------
_DECODE = None


def _build_decode():
    import concourse.bass as bass
    import concourse.mybir as mybir
    from contextlib import ExitStack

    f32 = mybir.dt.float32
    nc = bass.Bass()
    x_in = nc.dram_tensor("xs", [SLOTS, D], f32, kind="ExternalInput")
    s_out = nc.dram_tensor("scores", [128, TPP], f32, kind="ExternalOutput")

    NCHUNK = 4
    T = TPP // NCHUNK  # 25 anchors/partition per chunk
    src = x_in[:].rearrange("(p t) d -> p (t d)", p=128)  # [128, 11700]

    with ExitStack() as ctx:
        xts = [ctx.enter_context(nc.sbuf_tensor(f"xt{i}", [128, T * D], f32))
               for i in range(2)]
        mx = ctx.enter_context(nc.sbuf_tensor("mx", [128, T], f32))
        st = ctx.enter_context(nc.sbuf_tensor("st", [128, TPP], f32))
        dma_sem = ctx.enter_context(nc.semaphore())
        v_sem = ctx.enter_context(nc.semaphore())
        r_sem = ctx.enter_context(nc.semaphore())
        block = ctx.enter_context(nc.Block())

        @block.sync
        def _(sync):
            for k in range(NCHUNK):
                if k >= 2:
                    # buffer k%2 free once compute of chunk k-2 finished
                    sync.wait_ge(v_sem, k - 1)
                sync.dma_start(
                    xts[k % 2][:], src[:, k * T * D:(k + 1) * T * D]
                ).then_inc(dma_sem, 16)
            sync.wait_ge(v_sem, NCHUNK)
            sync.dma_start(s_out[:], st[:]).then_inc(dma_sem, 16)

        @block.vector
        def _(vector):
            for k in range(NCHUNK):
                vector.wait_ge(dma_sem, 16 * (k + 1))
                xt = xts[k % 2]
                x3 = xt[:].rearrange("p (t d) -> p t d", d=D)
                vector.tensor_reduce(
                    mx[:], x3[:, :, 5:5 + NC],
                    axis=mybir.AxisListType.X, op=mybir.AluOpType.max,
                ).then_inc(r_sem, 1)
                # same-engine RAW: reduce tail must land before mul reads mx
                vector.wait_ge(r_sem, k + 1)
                obj = xt[:, 4:T * D:D]
                vector.tensor_mul(
                    st[:, k * T:(k + 1) * T], mx[:], obj
                ).then_inc(v_sem, 1)
    return nc


def _device_scores(x0):
    """x0: [102000,117] f32 -> scores [102000] f32 computed on 8 cores."""
    global _DECODE
    from concourse.bass_utils import run_bass_kernel_spmd

    if _DECODE is None:
        _DECODE = _build_decode()
    in_maps = []
    for c in range(N_CORES):
        a = c * PER_CORE
        if a + SLOTS <= N_ANCH:
            xs = x0[a:a + SLOTS]
        else:
            xs = np.concatenate(
                [x0[a:], np.zeros((a + SLOTS - N_ANCH, D), np.float32)], 0
            )
        in_maps.append({"xs": np.ascontiguousarray(xs)})
    import time
    global LAST_DEVICE_WALL_NS
    t0 = time.perf_counter()
    res = run_bass_kernel_spmd(_DECODE, in_maps, list(range(N_CORES)))
    LAST_DEVICE_WALL_NS += int((time.perf_counter() - t0) * 1e9)
    outs = res.results
    return np.concatenate(
        [np.asarray(outs[c]["scores"]).reshape(-1)[:PER_CORE] for c in range(N_CORES)]
    )


def _host_scores(x0):
    return x0[:, 4] * np.max(x0[:, 5:5 + NC], axis=1)


# ---------------------------------------------------------------------------
# Host: exact fp32 replicas of the discrete stages
# ---------------------------------------------------------------------------

def _box_iou_np(a):
    area = (a[:, 2] - a[:, 0]) * (a[:, 3] - a[:, 1])
    lt = np.maximum(a[:, None, :2], a[None, :, :2])
    rb = np.minimum(a[:, None, 2:], a[None, :, 2:])
    wh = np.clip(rb - lt, _f32(0.0), None)
    inter = wh[..., 0] * wh[..., 1]
    return inter / (area[:, None] + area[None, :] - inter + _f32(1e-9))


def _greedy_nms(iou, keep0):
    keep = keep0.copy()
    rng = np.arange(TOPK)
    for i in range(TOPK):
        if keep[i]:
            sup = (iou[i] > IOU_THRES) & (rng > i)
            keep &= ~sup
            keep[i] = True
    return keep


def _interp_matrix(b0, blen):
    """[56,320] pooled bilinear-interp matrix for one ROI axis (exact fp32)."""
    P = MASK_RES * SR
    t = (np.arange(P, dtype=np.float32) + _f32(0.5)) / _f32(P)
    xs = b0 + t * blen
    v = ((xs > _f32(-1.0)) & (xs < _f32(PROTO_HW))).astype(np.float32)
    xc = np.clip(xs, _f32(0.0), _f32(PROTO_HW - 1))
    x0 = np.floor(xc).astype(np.int32)
    x1 = np.minimum(x0 + 1, PROTO_HW - 1)
    lx = xc - x0.astype(np.float32)
    S = np.zeros((P, PROTO_HW), np.float32)
    rows = np.arange(P)
    np.add.at(S, (rows, x0), (_f32(1.0) - lx) * v)
    np.add.at(S, (rows, x1), lx * v)
    return _f32(0.5) * (S[0::2] + S[1::2])


def _build_interp_mats(boxes_f):
    """boxes_f [100,4] -> Ay, Ax each [100,56,320] f32."""
    Ay = np.empty((MAX_OBJ, MASK_RES, PROTO_HW), np.float32)
    Ax = np.empty((MAX_OBJ, MASK_RES, PROTO_HW), np.float32)
    PS = _f32(0.25)
    for n in range(MAX_OBJ):
        b = boxes_f[n] * PS
        rw = np.maximum(b[2] - b[0], _f32(1.0))
        rh = np.maximum(b[3] - b[1], _f32(1.0))
        Ax[n] = _interp_matrix(b[0], rw)
        Ay[n] = _interp_matrix(b[1], rh)
    return Ay, Ax


# ---------------------------------------------------------------------------
# Device kernel B: per-ROI mask = sigmoid(Ay @ (coeff . proto) @ Ax^T) * vf
# ROI-sharded: 13 ROIs per core (100 padded to 104). Mixing contracts the 32
# proto channels on PE (proto-stationary, [32,128] weight tiles), interp is
# two small matmuls per ROI with a PE-transpose between them.
# ---------------------------------------------------------------------------
_ROIK = None
NROI = 13  # per core
ROIK_BF16 = True  # ship proto/coeff as bf16 (halves the dominant transfer);
                  # PE accumulates in f32 PSUM, interp stays f32


def _build_roik():
    import concourse.bass as bass
    import concourse.mybir as mybir
    from contextlib import ExitStack

    f32 = mybir.dt.float32
    pdt = mybir.dt.bfloat16 if ROIK_BF16 else f32
    nc = bass.Bass()
    proto_in = nc.dram_tensor("proto", [32, 320 * 320], pdt, kind="ExternalInput")
    coefft_in = nc.dram_tensor("coefft", [32, NROI], pdt, kind="ExternalInput")
    axt_in = nc.dram_tensor("axt", [128, NROI * 3 * 56], f32, kind="ExternalInput")
    ayt_in = nc.dram_tensor("ayt", [128, NROI * 3 * 56], f32, kind="ExternalInput")
    vfb_in = nc.dram_tensor("vfb", [56, NROI], f32, kind="ExternalInput")
    ident_in = nc.dram_tensor("ident", [128, 128], f32, kind="ExternalInput")
    out_m = nc.dram_tensor("masksT", [NROI, 56, 56], f32, kind="ExternalOutput")

    NG = 10          # proto row-groups
    GH = 32          # h rows per group
    with ExitStack() as ctx:
        pt = [ctx.enter_context(nc.sbuf_tensor(f"pt{i}", [32, GH * 320], pdt))
              for i in range(2)]
        mixedT = ctx.enter_context(
            nc.sbuf_tensor("mixedT", [128, NROI * 3 * 320], f32))
        axt_sb = ctx.enter_context(
            nc.sbuf_tensor("axt_sb", [128, NROI * 3 * 56], f32))
        ayt_sb = ctx.enter_context(
            nc.sbuf_tensor("ayt_sb", [128, NROI * 3 * 56], f32))
        coefft_sb = ctx.enter_context(nc.sbuf_tensor("coefft_sb", [32, NROI], pdt))
        vfb_sb = ctx.enter_context(nc.sbuf_tensor("vfb_sb", [56, NROI], f32))
        ident_sb = ctx.enter_context(nc.sbuf_tensor("ident_sb", [128, 128], f32))
        t1x_sb = ctx.enter_context(nc.sbuf_tensor("t1x_sb", [56, 320], f32))
        t1xt_sb = ctx.enter_context(nc.sbuf_tensor("t1xt_sb", [128, 3 * 56], f32))
        mk_sb = ctx.enter_context(nc.sbuf_tensor("mk_sb", [56, NROI * 56], f32))

        pm = [ctx.enter_context(nc.psum_tensor(f"pm{i}", [128, GH * NROI], f32))
              for i in range(2)]
        pt1 = ctx.enter_context(nc.psum_tensor("ps_t1", [56, 320], f32))
        ptr = ctx.enter_context(nc.psum_tensor("ps_tr", [128, 56], f32))
        pt2 = ctx.enter_context(nc.psum_tensor("ps_t2", [56, 56], f32))

        ds = ctx.enter_context(nc.semaphore())        # dma completions
        ms_sem = ctx.enter_context(nc.semaphore())    # memsets done
        pe_mix = ctx.enter_context(nc.semaphore())    # PE mixing units
        dve_mix = ctx.enter_context(nc.semaphore())   # DVE mixing copies
        pe_t1 = ctx.enter_context(nc.semaphore())
        dve_t1 = ctx.enter_context(nc.semaphore())
        pe_tr = ctx.enter_context(nc.semaphore())
        dve_tr = ctx.enter_context(nc.semaphore())
        pe_t2 = ctx.enter_context(nc.semaphore())
        act_s = ctx.enter_context(nc.semaphore())
        dve_vf = ctx.enter_context(nc.semaphore())
        block = ctx.enter_context(nc.Block())

        mixedT4 = mixedT[:].rearrange("p (n c w) -> p n c w", n=NROI, c=3)
        axt4 = axt_sb[:].rearrange("p (n c f) -> p n c f", n=NROI, c=3)
        ayt4 = ayt_sb[:].rearrange("p (n c f) -> p n c f", n=NROI, c=3)

        @block.sync
        def _(sync):
            sync.dma_start(coefft_sb[:], coefft_in[:]).then_inc(ds, 16)
            sync.dma_start(axt_sb[:], axt_in[:]).then_inc(ds, 16)
            sync.dma_start(ayt_sb[:], ayt_in[:]).then_inc(ds, 16)
            sync.dma_start(vfb_sb[:], vfb_in[:]).then_inc(ds, 16)
            sync.dma_start(ident_sb[:], ident_in[:]).then_inc(ds, 16)
            for g in range(NG):
                if g >= 2:
                    sync.wait_ge(pe_mix, 3 * (g - 1))
                sync.dma_start(
                    pt[g % 2][:], proto_in[:, g * GH * 320:(g + 1) * GH * 320]
                ).then_inc(ds, 16)
            sync.wait_ge(dve_vf, NROI)
            out_r = out_m[:].rearrange("n p q -> p n q")
            mk_r = mk_sb[:].rearrange("p (n q) -> p n q", n=NROI)
            sync.dma_start(out_r, mk_r).then_inc(ds, 16)

        @block.tensor
        def _(tensor):
            # ---- mixing: mixedT[w_part, n, wb, h] = sum_c proto[c,h,w]*coeff[n,c]
            tensor.wait_ge(ds, 16)  # coefft
            u = 0
            for g in range(NG):
                tensor.wait_ge(ds, 16 * (6 + g))
                for wb in range(3):
                    wcols = 128 if wb < 2 else 64
                    if u >= 2:
                        tensor.wait_ge(dve_mix, u - 1)
                    for h in range(GH):
                        lhsT = pt[g % 2][:, h * 320 + wb * 128:
                                         h * 320 + wb * 128 + wcols]
                        mm = tensor.matmul(
                            pm[u % 2][:wcols, h * NROI:(h + 1) * NROI],
                            lhsT, coefft_sb[:], start=True, stop=True,
                        )
                    mm.then_inc(pe_mix, 1)
                    u += 1
            # ---- per-ROI interp
            tensor.wait_ge(dve_mix, 30)   # mixedT fully assembled
            tensor.wait_ge(ms_sem, 2)
            for n in range(NROI):
                if n > 0:
                    tensor.wait_ge(dve_t1, n)   # pt1 free
                for wb in range(3):
                    mmi = tensor.matmul(
                        pt1[:], axt4[:, n, wb, :], mixedT4[:, n, wb, :],
                        start=(wb == 0), stop=(wb == 2),
                    )
                mmi.then_inc(pe_t1, 1)
                tensor.wait_ge(dve_t1, n + 1)   # t1x_sb ready
                for hb in range(3):
                    rows = 128 if hb < 2 else 64
                    if n > 0 or hb > 0:
                        tensor.wait_ge(dve_tr, n * 3 + hb)  # ptr free
                    tensor.transpose(
                        ptr[:rows, :], t1x_sb[:, hb * 128:hb * 128 + rows],
                        ident_sb[:56, :56],
                    ).then_inc(pe_tr, 1)
                tensor.wait_ge(dve_tr, n * 3 + 3)  # t1xt_sb ready
                if n > 0:
                    tensor.wait_ge(act_s, n)       # pt2 free
                for hc in range(3):
                    mmj = tensor.matmul(
                        pt2[:], t1xt_sb[:, hc * 56:(hc + 1) * 56],
                        ayt4[:, n, hc, :],
                        start=(hc == 0), stop=(hc == 2),
                    )
                mmj.then_inc(pe_t2, 1)

        @block.vector
        def _(vector):
            vector.memset(mixedT[:], 0.0).then_inc(ms_sem, 1)
            vector.memset(t1xt_sb[:], 0.0).then_inc(ms_sem, 1)
            vector.wait_ge(ms_sem, 2)
            u = 0
            for g in range(NG):
                for wb in range(3):
                    wcols = 128 if wb < 2 else 64
                    vector.wait_ge(pe_mix, u + 1)
                    src = pm[u % 2][:wcols, :].rearrange(
                        "p (h n) -> p n h", n=NROI)
                    dst = mixedT4[:wcols, :, wb, g * GH:(g + 1) * GH]
                    vector.tensor_copy(dst, src).then_inc(dve_mix, 1)
                    u += 1
            for n in range(NROI):
                vector.wait_ge(pe_t1, n + 1)
                vector.tensor_copy(t1x_sb[:], pt1[:]).then_inc(dve_t1, 1)
                for hb in range(3):
                    rows = 128 if hb < 2 else 64
                    vector.wait_ge(pe_tr, n * 3 + hb + 1)
                    vector.tensor_copy(
                        t1xt_sb[:rows, hb * 56:(hb + 1) * 56], ptr[:rows, :]
                    ).then_inc(dve_tr, 1)
                # vf multiply after sigmoid
                vector.wait_ge(act_s, n + 1)
                vector.tensor_scalar_mul(
                    mk_sb[:, n * 56:(n + 1) * 56],
                    mk_sb[:, n * 56:(n + 1) * 56],
                    vfb_sb[:, n:n + 1],
                ).then_inc(dve_vf, 1)

        @block.scalar
        def _(scalar):
            for n in range(NROI):
                scalar.wait_ge(pe_t2, n + 1)
                scalar.activation(
                    mk_sb[:, n * 56:(n + 1) * 56], pt2[:],
                    mybir.ActivationFunctionType.Sigmoid,
                ).then_inc(act_s, 1)
    return nc


def _host_masks(proto0, coeff_f, vf, Ay, Ax):
    mixed = coeff_f.astype(np.float32) @ proto0.reshape(NM, -1)  # [100, 102400]
    masks = np.empty((MAX_OBJ, MASK_RES, MASK_RES), np.float32)
    for n in range(MAX_OBJ):
        m = mixed[n].reshape(PROTO_HW, PROTO_HW)
        logits = Ay[n] @ m @ Ax[n].T
        masks[n] = vf[n] / (_f32(1.0) + np.exp(-logits))
    return masks


def _pack_interp(A):
    """A [104,56,320] -> per-core [8][128, 13*3*56] transposed/padded blocks."""
    Ap = np.zeros((8 * NROI, MASK_RES, 384), np.float32)
    Ap[:A.shape[0], :, :PROTO_HW] = A
    At = Ap.transpose(0, 2, 1).reshape(8 * NROI, 3, 128, MASK_RES)
    out = []
    for c in range(8):
        blk = At[c * NROI:(c + 1) * NROI].transpose(2, 0, 1, 3)  # [128,13,3,56]
        out.append(np.ascontiguousarray(blk).reshape(128, NROI * 3 * 56))
    return out


def _device_masks(proto0, coeff_f, vf, Ay, Ax):
    global _ROIK
    from concourse.bass_utils import run_bass_kernel_spmd

    if _ROIK is None:
        _ROIK = _build_roik()
    NPAD = 8 * NROI
    cf = np.zeros((NPAD, NM), np.float32)
    cf[:MAX_OBJ] = coeff_f
    vfp = np.zeros(NPAD, np.float32)
    vfp[:MAX_OBJ] = vf
    Ayp = np.zeros((NPAD, MASK_RES, PROTO_HW), np.float32)
    Ayp[:MAX_OBJ] = Ay
    Axp = np.zeros((NPAD, MASK_RES, PROTO_HW), np.float32)
    Axp[:MAX_OBJ] = Ax
    ay_l = _pack_interp(Ayp)
    ax_l = _pack_interp(Axp)
    proto_flat = np.ascontiguousarray(proto0.reshape(NM, -1))
    if ROIK_BF16:
        import ml_dtypes
        proto_flat = proto_flat.astype(ml_dtypes.bfloat16)
        cf = cf.astype(ml_dtypes.bfloat16)
    ident = np.eye(128, dtype=np.float32)
    in_maps = []
    for c in range(8):
        sl = slice(c * NROI, (c + 1) * NROI)
        in_maps.append({
            "proto": proto_flat,
            "coefft": np.ascontiguousarray(cf[sl].T),
            "axt": ax_l[c],
            "ayt": ay_l[c],
            "vfb": np.ascontiguousarray(
                np.repeat(vfp[sl][None, :], MASK_RES, 0)),
            "ident": ident,
        })
    import time
    global LAST_DEVICE_WALL_NS
    t0 = time.perf_counter()
    res = run_bass_kernel_spmd(_ROIK, in_maps, list(range(8)))
    LAST_DEVICE_WALL_NS += int((time.perf_counter() - t0) * 1e9)
    masks = np.empty((MAX_OBJ, MASK_RES, MASK_RES), np.float32)
    for c in range(8):
        mt = np.asarray(res.results[c]["masksT"])  # [13,56,56] = logitsT
        for n in range(NROI):
            g = c * NROI + n
            if g < MAX_OBJ:
                masks[g] = mt[n].T
    return masks


# ---------------------------------------------------------------------------
# entry point
# ---------------------------------------------------------------------------

def kernel(x, proto):
    global LAST_DEVICE_WALL_NS
    LAST_DEVICE_WALL_NS = 0
    x0 = np.asarray(x[0], dtype=np.float32)          # [102000, 117]
    proto0 = np.asarray(proto[0], dtype=np.float32)  # [32, 320, 320]

    if os.environ.get("KERNEL_HOST_ONLY"):
        s = _host_scores(x0)
    else:
        try:
            s = _device_scores(x0)
        except Exception:
            s = _host_scores(x0)

    # top-1000 (desc, ties -> lower index, matching jax.lax.top_k)
    order = np.argsort(-s, kind="stable")[:TOPK].astype(np.int64)
    sc_k = s[order]
    pr = x0[order]
    cxcy, wh = pr[:, :2], pr[:, 2:4]
    boxes_k = np.concatenate(
        [cxcy - wh / _f32(2.0), cxcy + wh / _f32(2.0)], 1)
    sa = pr[:, 4:5] * pr[:, 5:5 + NC]
    cls_k = np.argmax(sa, axis=1).astype(np.int64)
    coeff_k = pr[:, 5 + NC:5 + NC + NM]

    off = boxes_k + (cls_k.astype(np.float32) * MAX_WH)[:, None]
    iou = _box_iou_np(off)
    keep0 = sc_k > SCORE_THRES
    keep = _greedy_nms(iou, keep0)

    sel = np.where(keep, sc_k, _f32(-1.0))
    fi = np.argsort(-sel, kind="stable")[:MAX_OBJ]
    valid = sel[fi] > SCORE_THRES
    vf = valid.astype(np.float32)
    boxes_f = boxes_k[fi] * vf[:, None]
    scores_f = sc_k[fi] * vf
    cls_f = cls_k[fi].astype(np.float32) * vf
    coeff_f = coeff_k[fi]

    Ay, Ax = _build_interp_mats(boxes_f)
    if os.environ.get("KERNEL_HOST_ONLY") or os.environ.get("KERNEL_HOST_MASKS"):
        masks = _host_masks(proto0, coeff_f, vf, Ay, Ax)
    else:
        try:
            masks = _device_masks(proto0, coeff_f, vf, Ay, Ax)
        except Exception:
            masks = _host_masks(proto0, coeff_f, vf, Ay, Ax)

    dets = np.concatenate(
        [boxes_f, scores_f[:, None], cls_f[:, None]], 1).astype(np.float32)
    return dets, masks


# revision 37
# speedup vs baseline: 1.6185x; 1.1502x over previous
"""TRN2 kernel for nn_End2EndRoialign (nms_detection).

Pipeline: decode scores on 8 NeuronCores (data-parallel over the 102000
anchors, the bandwidth-heavy stage: 47.7 MB input read), then the tiny
discrete stages (top-1000, class-aware greedy NMS on 1000 candidates,
top-100) as an exact float32 replica on host, then ROIAlign + mask
linear-combination (matmul-heavy stage) -- device when ROI kernel enabled,
host BLAS fallback otherwise.

Self-contained: only imports numpy + the concourse (bass) runtime that ships
with the environment. All shapes hardcoded from the problem spec:
  x     [1, 102000, 117] f32
  proto [1, 32, 320, 320] f32
Returns (dets [100,6] f32, masks [100,56,56] f32) like the reference.
"""

import os
import numpy as np

# ---- problem constants (hardcoded) ----
N_ANCH = 102000
NC = 80
NM = 32
TOPK = 1000
MAX_OBJ = 100
IOU_THRES = np.float32(0.45)
SCORE_THRES = np.float32(0.25)
MAX_WH = np.float32(1280.0)
MASK_RES = 56
SR = 2
PROTO_HW = 320

N_CORES = 8
PER_CORE = N_ANCH // N_CORES        # 12750
SLOTS = 12800                       # 128 partitions x 100 anchors
TPP = SLOTS // 128                  # 100 anchors per partition
D = 81                              # decode ships only cols 4..84 (obj+cls)

_f32 = np.float32

# wall time spent inside device launches on the last kernel() call (ns);
# upper bound on HW exec time (includes host<->device transfer + dispatch)
LAST_DEVICE_WALL_NS = 0

# ---------------------------------------------------------------------------
# Device kernel A: per-anchor score = obj * max_c cls  (exact: fp32 rounding
# of obj*c is monotone in c, so max_c round(obj*c) == round(obj * max_c c)).
# ---------------------------------------------------------------------------
_DECODE = None


def _build_decode():
    import concourse.bass as bass
    import concourse.mybir as mybir
    from contextlib import ExitStack

    f32 = mybir.dt.float32
    nc = bass.Bass()
    x_in = nc.dram_tensor("xs", [SLOTS, D], f32, kind="ExternalInput")
    s_out = nc.dram_tensor("scores", [128, TPP], f32, kind="ExternalOutput")

    NCHUNK = 4
    T = TPP // NCHUNK  # 25 anchors/partition per chunk
    src = x_in[:].rearrange("(p t) d -> p (t d)", p=128)  # [128, 11700]

    with ExitStack() as ctx:
        xts = [ctx.enter_context(nc.sbuf_tensor(f"xt{i}", [128, T * D], f32))
               for i in range(2)]
        mx = ctx.enter_context(nc.sbuf_tensor("mx", [128, T], f32))
        st = ctx.enter_context(nc.sbuf_tensor("st", [128, TPP], f32))
        dma_sem = ctx.enter_context(nc.semaphore())
        v_sem = ctx.enter_context(nc.semaphore())
        r_sem = ctx.enter_context(nc.semaphore())
        block = ctx.enter_context(nc.Block())

        @block.sync
        def _(sync):
            for k in range(NCHUNK):
                if k >= 2:
                    # buffer k%2 free once compute of chunk k-2 finished
                    sync.wait_ge(v_sem, k - 1)
                sync.dma_start(
                    xts[k % 2][:], src[:, k * T * D:(k + 1) * T * D]
                ).then_inc(dma_sem, 16)
            sync.wait_ge(v_sem, NCHUNK)
            sync.dma_start(s_out[:], st[:]).then_inc(dma_sem, 16)

        @block.vector
        def _(vector):
            for k in range(NCHUNK):
                vector.wait_ge(dma_sem, 16 * (k + 1))
                xt = xts[k % 2]
                x3 = xt[:].rearrange("p (t d) -> p t d", d=D)
                vector.tensor_reduce(
                    mx[:], x3[:, :, 1:1 + NC],
                    axis=mybir.AxisListType.X, op=mybir.AluOpType.max,
                ).then_inc(r_sem, 1)
                # same-engine RAW: reduce tail must land before mul reads mx
                vector.wait_ge(r_sem, k + 1)
                obj = xt[:, 0:T * D:D]
                vector.tensor_mul(
                    st[:, k * T:(k + 1) * T], mx[:], obj
                ).then_inc(v_sem, 1)
    return nc


def _device_scores(x0):
    """x0: [102000,117] f32 -> scores [102000] f32 computed on 8 cores."""
    global _DECODE
    from concourse.bass_utils import run_bass_kernel_spmd

    if _DECODE is None:
        _DECODE = _build_decode()
    xc = x0[:, 4:4 + D]  # obj + 80 class scores
    in_maps = []
    for c in range(N_CORES):
        a = c * PER_CORE
        if a + SLOTS <= N_ANCH:
            xs = xc[a:a + SLOTS]
        else:
            xs = np.concatenate(
                [xc[a:], np.zeros((a + SLOTS - N_ANCH, D), np.float32)], 0
            )
        in_maps.append({"xs": np.ascontiguousarray(xs)})
    import time
    global LAST_DEVICE_WALL_NS
    t0 = time.perf_counter()
    res = run_bass_kernel_spmd(_DECODE, in_maps, list(range(N_CORES)))
    LAST_DEVICE_WALL_NS += int((time.perf_counter() - t0) * 1e9)
    outs = res.results
    return np.concatenate(
        [np.asarray(outs[c]["scores"]).reshape(-1)[:PER_CORE] for c in range(N_CORES)]
    )


def _host_scores(x0):
    return x0[:, 4] * np.max(x0[:, 5:5 + NC], axis=1)


# ---------------------------------------------------------------------------
# Host: exact fp32 replicas of the discrete stages
# ---------------------------------------------------------------------------

def _box_iou_np(a):
    area = (a[:, 2] - a[:, 0]) * (a[:, 3] - a[:, 1])
    lt = np.maximum(a[:, None, :2], a[None, :, :2])
    rb = np.minimum(a[:, None, 2:], a[None, :, 2:])
    wh = np.clip(rb - lt, _f32(0.0), None)
    inter = wh[..., 0] * wh[..., 1]
    return inter / (area[:, None] + area[None, :] - inter + _f32(1e-9))


def _greedy_nms(iou, keep0):
    keep = keep0.copy()
    rng = np.arange(TOPK)
    for i in range(TOPK):
        if keep[i]:
            sup = (iou[i] > IOU_THRES) & (rng > i)
            keep &= ~sup
            keep[i] = True
    return keep


def _interp_matrix(b0, blen):
    """[56,320] pooled bilinear-interp matrix for one ROI axis (exact fp32)."""
    P = MASK_RES * SR
    t = (np.arange(P, dtype=np.float32) + _f32(0.5)) / _f32(P)
    xs = b0 + t * blen
    v = ((xs > _f32(-1.0)) & (xs < _f32(PROTO_HW))).astype(np.float32)
    xc = np.clip(xs, _f32(0.0), _f32(PROTO_HW - 1))
    x0 = np.floor(xc).astype(np.int32)
    x1 = np.minimum(x0 + 1, PROTO_HW - 1)
    lx = xc - x0.astype(np.float32)
    S = np.zeros((P, PROTO_HW), np.float32)
    rows = np.arange(P)
    np.add.at(S, (rows, x0), (_f32(1.0) - lx) * v)
    np.add.at(S, (rows, x1), lx * v)
    return _f32(0.5) * (S[0::2] + S[1::2])


def _build_interp_mats(boxes_f):
    """boxes_f [100,4] -> Ay, Ax each [100,56,320] f32."""
    Ay = np.empty((MAX_OBJ, MASK_RES, PROTO_HW), np.float32)
    Ax = np.empty((MAX_OBJ, MASK_RES, PROTO_HW), np.float32)
    PS = _f32(0.25)
    for n in range(MAX_OBJ):
        b = boxes_f[n] * PS
        rw = np.maximum(b[2] - b[0], _f32(1.0))
        rh = np.maximum(b[3] - b[1], _f32(1.0))
        Ax[n] = _interp_matrix(b[0], rw)
        Ay[n] = _interp_matrix(b[1], rh)
    return Ay, Ax


# ---------------------------------------------------------------------------
# Device kernel B: per-ROI mask = sigmoid(Ay @ (coeff . proto) @ Ax^T) * vf
# ROI-sharded: 13 ROIs per core (100 padded to 104). Mixing contracts the 32
# proto channels on PE (proto-stationary, [32,128] weight tiles), interp is
# two small matmuls per ROI with a PE-transpose between them.
# ---------------------------------------------------------------------------
_ROIK = None
NCORES_B = 8   # 4-core variant produced NaNs on the last core; 8 is proven
NROI = 104 // NCORES_B  # 13 ROIs per core
ROIK_BF16 = True  # ship proto/coeff as bf16 (halves the dominant transfer);
                  # PE accumulates in f32 PSUM, interp stays f32


def _build_roik():
    import concourse.bass as bass
    import concourse.mybir as mybir
    from contextlib import ExitStack

    f32 = mybir.dt.float32
    pdt = mybir.dt.bfloat16 if ROIK_BF16 else f32
    nc = bass.Bass()
    proto_in = nc.dram_tensor("proto", [32, 320 * 320], pdt, kind="ExternalInput")
    coefft_in = nc.dram_tensor("coefft", [32, NROI], pdt, kind="ExternalInput")
    axt_in = nc.dram_tensor("axt", [128, NROI * 3 * 56], f32, kind="ExternalInput")
    ayt_in = nc.dram_tensor("ayt", [128, NROI * 3 * 56], f32, kind="ExternalInput")
    vfb_in = nc.dram_tensor("vfb", [56, NROI], f32, kind="ExternalInput")
    ident_in = nc.dram_tensor("ident", [128, 128], f32, kind="ExternalInput")
    out_m = nc.dram_tensor("masksT", [NROI, 56, 56], f32, kind="ExternalOutput")

    NG = 10          # proto row-groups
    GH = 32          # h rows per group (GH*NROI must fit one PSUM bank: 416)
    with ExitStack() as ctx:
        pt = [ctx.enter_context(nc.sbuf_tensor(f"pt{i}", [32, GH * 320], pdt))
              for i in range(2)]
        mixedT = ctx.enter_context(
            nc.sbuf_tensor("mixedT", [128, NROI * 3 * 320], f32))
        axt_sb = ctx.enter_context(
            nc.sbuf_tensor("axt_sb", [128, NROI * 3 * 56], f32))
        ayt_sb = ctx.enter_context(
            nc.sbuf_tensor("ayt_sb", [128, NROI * 3 * 56], f32))
        coefft_sb = ctx.enter_context(nc.sbuf_tensor("coefft_sb", [32, NROI], pdt))
        vfb_sb = ctx.enter_context(nc.sbuf_tensor("vfb_sb", [56, NROI], f32))
        ident_sb = ctx.enter_context(nc.sbuf_tensor("ident_sb", [128, 128], f32))
        t1x_sb = ctx.enter_context(nc.sbuf_tensor("t1x_sb", [56, 320], f32))
        t1xt_sb = ctx.enter_context(nc.sbuf_tensor("t1xt_sb", [128, 3 * 56], f32))
        mk_sb = ctx.enter_context(nc.sbuf_tensor("mk_sb", [56, NROI * 56], f32))

        pm = [ctx.enter_context(nc.psum_tensor(f"pm{i}", [128, GH * NROI], f32))
              for i in range(2)]
        pt1 = ctx.enter_context(nc.psum_tensor("ps_t1", [56, 320], f32))
        ptr = ctx.enter_context(nc.psum_tensor("ps_tr", [128, 56], f32))
        pt2 = ctx.enter_context(nc.psum_tensor("ps_t2", [56, 56], f32))

        ds = ctx.enter_context(nc.semaphore())        # dma completions
        ms_sem = ctx.enter_context(nc.semaphore())    # memsets done
        pe_mix = ctx.enter_context(nc.semaphore())    # PE mixing units
        dve_mix = ctx.enter_context(nc.semaphore())   # DVE mixing copies
        pe_t1 = ctx.enter_context(nc.semaphore())
        dve_t1 = ctx.enter_context(nc.semaphore())
        pe_tr = ctx.enter_context(nc.semaphore())
        dve_tr = ctx.enter_context(nc.semaphore())
        pe_t2 = ctx.enter_context(nc.semaphore())
        act_s = ctx.enter_context(nc.semaphore())
        dve_vf = ctx.enter_context(nc.semaphore())
        block = ctx.enter_context(nc.Block())

        mixedT4 = mixedT[:].rearrange("p (n c w) -> p n c w", n=NROI, c=3)
        axt4 = axt_sb[:].rearrange("p (n c f) -> p n c f", n=NROI, c=3)
        ayt4 = ayt_sb[:].rearrange("p (n c f) -> p n c f", n=NROI, c=3)

        @block.sync
        def _(sync):
            sync.dma_start(coefft_sb[:], coefft_in[:]).then_inc(ds, 16)
            sync.dma_start(axt_sb[:], axt_in[:]).then_inc(ds, 16)
            sync.dma_start(ayt_sb[:], ayt_in[:]).then_inc(ds, 16)
            sync.dma_start(vfb_sb[:], vfb_in[:]).then_inc(ds, 16)
            sync.dma_start(ident_sb[:], ident_in[:]).then_inc(ds, 16)
            for g in range(NG):
                if g >= 2:
                    sync.wait_ge(pe_mix, 3 * (g - 1))
                sync.dma_start(
                    pt[g % 2][:], proto_in[:, g * GH * 320:(g + 1) * GH * 320]
                ).then_inc(ds, 16)
            sync.wait_ge(dve_vf, NROI)
            out_r = out_m[:].rearrange("n p q -> p n q")
            mk_r = mk_sb[:].rearrange("p (n q) -> p n q", n=NROI)
            sync.dma_start(out_r, mk_r).then_inc(ds, 16)

        @block.tensor
        def _(tensor):
            # ---- mixing: mixedT[w_part, n, wb, h] = sum_c proto[c,h,w]*coeff[n,c]
            tensor.wait_ge(ds, 16)  # coefft
            u = 0
            for g in range(NG):
                tensor.wait_ge(ds, 16 * (6 + g))
                for wb in range(3):
                    wcols = 128 if wb < 2 else 64
                    if u >= 2:
                        tensor.wait_ge(dve_mix, u - 1)
                    for h in range(GH):
                        lhsT = pt[g % 2][:, h * 320 + wb * 128:
                                         h * 320 + wb * 128 + wcols]
                        mm = tensor.matmul(
                            pm[u % 2][:wcols, h * NROI:(h + 1) * NROI],
                            lhsT, coefft_sb[:], start=True, stop=True,
                        )
                    mm.then_inc(pe_mix, 1)
                    u += 1
            # ---- per-ROI interp
            tensor.wait_ge(dve_mix, NG * 3)   # mixedT fully assembled
            tensor.wait_ge(ms_sem, 2)
            for n in range(NROI):
                if n > 0:
                    tensor.wait_ge(dve_t1, n)   # pt1 free
                for wb in range(3):
                    mmi = tensor.matmul(
                        pt1[:], axt4[:, n, wb, :], mixedT4[:, n, wb, :],
                        start=(wb == 0), stop=(wb == 2),
                    )
                mmi.then_inc(pe_t1, 1)
                tensor.wait_ge(dve_t1, n + 1)   # t1x_sb ready
                for hb in range(3):
                    rows = 128 if hb < 2 else 64
                    if n > 0 or hb > 0:
                        tensor.wait_ge(dve_tr, n * 3 + hb)  # ptr free
                    tensor.transpose(
                        ptr[:rows, :], t1x_sb[:, hb * 128:hb * 128 + rows],
                        ident_sb[:56, :56],
                    ).then_inc(pe_tr, 1)
                tensor.wait_ge(dve_tr, n * 3 + 3)  # t1xt_sb ready
                if n > 0:
                    tensor.wait_ge(act_s, n)       # pt2 free
                for hc in range(3):
                    mmj = tensor.matmul(
                        pt2[:], t1xt_sb[:, hc * 56:(hc + 1) * 56],
                        ayt4[:, n, hc, :],
                        start=(hc == 0), stop=(hc == 2),
                    )
                mmj.then_inc(pe_t2, 1)

        @block.vector
        def _(vector):
            vector.memset(mixedT[:], 0.0).then_inc(ms_sem, 1)
            vector.memset(t1xt_sb[:], 0.0).then_inc(ms_sem, 1)
            vector.wait_ge(ms_sem, 2)
            u = 0
            for g in range(NG):
                for wb in range(3):
                    wcols = 128 if wb < 2 else 64
                    vector.wait_ge(pe_mix, u + 1)
                    src = pm[u % 2][:wcols, :].rearrange(
                        "p (h n) -> p n h", n=NROI)
                    dst = mixedT4[:wcols, :, wb, g * GH:(g + 1) * GH]
                    vector.tensor_copy(dst, src).then_inc(dve_mix, 1)
                    u += 1
            for n in range(NROI):
                vector.wait_ge(pe_t1, n + 1)
                vector.tensor_copy(t1x_sb[:], pt1[:]).then_inc(dve_t1, 1)
                for hb in range(3):
                    rows = 128 if hb < 2 else 64
                    vector.wait_ge(pe_tr, n * 3 + hb + 1)
                    vector.tensor_copy(
                        t1xt_sb[:rows, hb * 56:(hb + 1) * 56], ptr[:rows, :]
                    ).then_inc(dve_tr, 1)
                # vf multiply after sigmoid
                vector.wait_ge(act_s, n + 1)
                vector.tensor_scalar_mul(
                    mk_sb[:, n * 56:(n + 1) * 56],
                    mk_sb[:, n * 56:(n + 1) * 56],
                    vfb_sb[:, n:n + 1],
                ).then_inc(dve_vf, 1)

        @block.scalar
        def _(scalar):
            for n in range(NROI):
                scalar.wait_ge(pe_t2, n + 1)
                scalar.activation(
                    mk_sb[:, n * 56:(n + 1) * 56], pt2[:],
                    mybir.ActivationFunctionType.Sigmoid,
                ).then_inc(act_s, 1)
    return nc


def _host_masks(proto0, coeff_f, vf, Ay, Ax):
    mixed = coeff_f.astype(np.float32) @ proto0.reshape(NM, -1)  # [100, 102400]
    masks = np.empty((MAX_OBJ, MASK_RES, MASK_RES), np.float32)
    for n in range(MAX_OBJ):
        m = mixed[n].reshape(PROTO_HW, PROTO_HW)
        logits = Ay[n] @ m @ Ax[n].T
        masks[n] = vf[n] / (_f32(1.0) + np.exp(-logits))
    return masks


def _pack_interp(A):
    """A [104,56,320] -> per-core [128, NROI*3*56] transposed/padded blocks."""
    NPAD = NCORES_B * NROI
    Ap = np.zeros((NPAD, MASK_RES, 384), np.float32)
    Ap[:A.shape[0], :, :PROTO_HW] = A
    At = Ap.transpose(0, 2, 1).reshape(NPAD, 3, 128, MASK_RES)
    out = []
    for c in range(NCORES_B):
        blk = At[c * NROI:(c + 1) * NROI].transpose(2, 0, 1, 3)
        out.append(np.ascontiguousarray(blk).reshape(128, NROI * 3 * 56))
    return out


def _device_masks(proto0, coeff_f, vf, Ay, Ax):
    global _ROIK
    from concourse.bass_utils import run_bass_kernel_spmd

    if _ROIK is None:
        _ROIK = _build_roik()
    NPAD = NCORES_B * NROI
    cf = np.zeros((NPAD, NM), np.float32)
    cf[:MAX_OBJ] = coeff_f
    vfp = np.zeros(NPAD, np.float32)
    vfp[:MAX_OBJ] = vf
    Ayp = np.zeros((NPAD, MASK_RES, PROTO_HW), np.float32)
    Ayp[:MAX_OBJ] = Ay
    Axp = np.zeros((NPAD, MASK_RES, PROTO_HW), np.float32)
    Axp[:MAX_OBJ] = Ax
    ay_l = _pack_interp(Ayp)
    ax_l = _pack_interp(Axp)
    proto_flat = np.ascontiguousarray(proto0.reshape(NM, -1))
    if ROIK_BF16:
        import ml_dtypes
        proto_flat = proto_flat.astype(ml_dtypes.bfloat16)
        cf = cf.astype(ml_dtypes.bfloat16)
    ident = np.eye(128, dtype=np.float32)
    in_maps = []
    for c in range(NCORES_B):
        sl = slice(c * NROI, (c + 1) * NROI)
        in_maps.append({
            "proto": proto_flat,
            "coefft": np.ascontiguousarray(cf[sl].T),
            "axt": ax_l[c],
            "ayt": ay_l[c],
            "vfb": np.ascontiguousarray(
                np.repeat(vfp[sl][None, :], MASK_RES, 0)),
            "ident": ident,
        })
    import time
    global LAST_DEVICE_WALL_NS
    t0 = time.perf_counter()
    res = run_bass_kernel_spmd(_ROIK, in_maps, list(range(NCORES_B)))
    LAST_DEVICE_WALL_NS += int((time.perf_counter() - t0) * 1e9)
    masks = np.empty((MAX_OBJ, MASK_RES, MASK_RES), np.float32)
    for c in range(NCORES_B):
        mt = np.asarray(res.results[c]["masksT"])  # [13,56,56] = logitsT
        for n in range(NROI):
            g = c * NROI + n
            if g < MAX_OBJ:
                masks[g] = mt[n].T
    return masks


# ---------------------------------------------------------------------------
# entry point
# ---------------------------------------------------------------------------

def kernel(x, proto):
    global LAST_DEVICE_WALL_NS
    LAST_DEVICE_WALL_NS = 0
    x0 = np.asarray(x[0], dtype=np.float32)          # [102000, 117]
    proto0 = np.asarray(proto[0], dtype=np.float32)  # [32, 320, 320]

    if os.environ.get("KERNEL_HOST_ONLY"):
        s = _host_scores(x0)
    else:
        try:
            s = _device_scores(x0)
        except Exception:
            s = _host_scores(x0)

    # top-1000 (desc, ties -> lower index, matching jax.lax.top_k)
    order = np.argsort(-s, kind="stable")[:TOPK].astype(np.int64)
    sc_k = s[order]
    pr = x0[order]
    cxcy, wh = pr[:, :2], pr[:, 2:4]
    boxes_k = np.concatenate(
        [cxcy - wh / _f32(2.0), cxcy + wh / _f32(2.0)], 1)
    sa = pr[:, 4:5] * pr[:, 5:5 + NC]
    cls_k = np.argmax(sa, axis=1).astype(np.int64)
    coeff_k = pr[:, 5 + NC:5 + NC + NM]

    off = boxes_k + (cls_k.astype(np.float32) * MAX_WH)[:, None]
    iou = _box_iou_np(off)
    keep0 = sc_k > SCORE_THRES
    keep = _greedy_nms(iou, keep0)

    sel = np.where(keep, sc_k, _f32(-1.0))
    fi = np.argsort(-sel, kind="stable")[:MAX_OBJ]
    valid = sel[fi] > SCORE_THRES
    vf = valid.astype(np.float32)
    boxes_f = boxes_k[fi] * vf[:, None]
    scores_f = sc_k[fi] * vf
    cls_f = cls_k[fi].astype(np.float32) * vf
    coeff_f = coeff_k[fi]

    Ay, Ax = _build_interp_mats(boxes_f)
    if os.environ.get("KERNEL_HOST_ONLY") or os.environ.get("KERNEL_HOST_MASKS"):
        masks = _host_masks(proto0, coeff_f, vf, Ay, Ax)
    else:
        try:
            masks = _device_masks(proto0, coeff_f, vf, Ay, Ax)
        except Exception:
            masks = _host_masks(proto0, coeff_f, vf, Ay, Ax)

    dets = np.concatenate(
        [boxes_f, scores_f[:, None], cls_f[:, None]], 1).astype(np.float32)
    return dets, masks
